# revision 1
# baseline (speedup 1.0000x reference)
"""GQA kernel for Trainium2, 8-core tensor-parallel over KV groups.

Model: HIDDEN=2048, HEADS=32, GROUPS=8, HEAD_DIM=64, SEQ=2048, BSZ=1.
Core g owns KV group g: its 4 query heads (Wq cols), Wk/Wv col slice,
and Wo row slice. Each core computes a full [SEQ, HIDDEN] partial of the
output projection; the host sums the 8 partials and adds bo.

Device-side layout: everything transposed (hidden/head_dim on SBUF
partitions). Scores are computed as S^T [t, s] tiles; softmax needs no
max-subtraction (|scores| <~ 8); the softmax denominator comes for free
from an appended ones-column on V in the P^T @ V_aug matmul.
"""

import sys
import numpy as np

for _p in ("/opt/trn_rl_repo", "/root/.axon_site/_ro/trn_rl_repo"):
    if _p not in sys.path:
        sys.path.insert(0, _p)

import concourse.bass as bass
import concourse.tile as tile
from concourse import mybir
from concourse.bass_utils import run_bass_kernel_spmd

S = 2048          # sequence length
H = 2048          # hidden
G = 8             # kv groups == cores
R = 4             # query heads per group
D = 64            # head dim
HDG = R * D       # 256 per-core q width
KVW = 2 * D       # 128: concat(k, v) width
NT = S // 128     # 16 t-tiles
NC = S // 512     # 4 s-chunks
NH = H // 128     # 16 h-tiles
F32 = mybir.dt.float32

_PROGRAM_CACHE = {}


_WAIT_LIMITS = {}


def _split_excess_waits(nc, default_max=1):
    """walrus structs support a limited number of sem waits per
    instruction (TPB_CTRL: 4, matmul's S3_LW: 1). Move the excess onto
    NoOp carriers inserted just before, on the same engine."""
    idx = 0
    for fn in nc.m.functions:
        for blk in fn.blocks:
            insts = list(blk.instructions)
            out = []
            changed = False
            for inst in insts:
                max_waits = _WAIT_LIMITS.get(type(inst).__name__, default_max)
                si = inst.sync_info
                if si is not None and si.on_wait and len(si.on_wait) > max_waits:
                    waits = list(si.on_wait)
                    head, keep = waits[:-max_waits], waits[-max_waits:]
                    while head:
                        chunk, head = head[:max_waits], head[max_waits:]
                        nop = mybir.InstNoOp(
                            name=f"waitsplit-{idx}",
                            sync_info=mybir.SyncInfo(on_wait=chunk, on_update=[]),
                            engine=inst.engine,
                            bass_nofuse=True,
                        )
                        idx += 1
                        nc.register_instruction(nop)
                        out.append(nop)
                    si.on_wait = keep
                    inst.sync_info = si
                    changed = True
                out.append(inst)
            if changed:
                blk.instructions = out


def _build(causal: bool):
    """Build the SPMD program (same for all cores; data differs)."""
    nc = bass.Bass(trn_type="TRN2", target_bir_lowering=False, debug=False)
    f32r = mybir.dt.float32r

    xT = nc.dram_tensor("xT", [H, S], f32r, kind="ExternalInput").ap()
    wq = nc.dram_tensor("wq", [H, HDG], f32r, kind="ExternalInput").ap()
    bq = nc.dram_tensor("bq", [128, 2], F32, kind="ExternalInput").ap()
    wkv = nc.dram_tensor("wkv", [H, KVW], f32r, kind="ExternalInput").ap()
    bkv = nc.dram_tensor("bkv", [128, 1], F32, kind="ExternalInput").ap()
    wo = nc.dram_tensor("wo", [HDG, H], f32r, kind="ExternalInput").ap()
    mblk = nc.dram_tensor("mblk", [128, 128], F32, kind="ExternalInput").ap()
    ident = nc.dram_tensor("ident", [128, 64], f32r, kind="ExternalInput").ap()
    onesd = nc.dram_tensor("onesd", [128, 128], f32r, kind="ExternalInput").ap()
    if not causal:
        mfull = nc.dram_tensor("mfull", [S, S], mybir.dt.bfloat16,
                               kind="ExternalInput").ap()
    partial = nc.dram_tensor("partial", [S, H], F32, kind="ExternalOutput").ap()

    with tile.TileContext(nc) as tc:
        with (
            tc.tile_pool(name="wpool", bufs=1) as wpool,
            tc.tile_pool(name="big", bufs=1) as big,
            tc.tile_pool(name="xp", bufs=3) as xp,
            tc.tile_pool(name="ptp", bufs=3) as ptp,
            tc.tile_pool(name="rbp", bufs=2) as rbp,
            tc.tile_pool(name="opp", bufs=3) as opp,
            tc.tile_pool(name="mfp", bufs=8) as mfp,
            tc.tile_pool(name="ps2", bufs=3, space="PSUM") as ps2,
            tc.tile_pool(name="ps1", bufs=2, space="PSUM") as ps1,
        ):
            # ---- resident weights / constants ----
            wq_s = wpool.tile([128, NH, HDG], f32r)
            nc.sync.dma_start(out=wq_s, in_=wq.rearrange("(i p) c -> p i c", p=128))
            wkv_s = wpool.tile([128, NH, KVW], f32r)
            nc.sync.dma_start(out=wkv_s, in_=wkv.rearrange("(i p) c -> p i c", p=128))
            wo_s = wpool.tile([128, 2, H], f32r)
            bq_s = wpool.tile([128, 2], F32)
            nc.sync.dma_start(out=bq_s, in_=bq)
            bkv_s = wpool.tile([128, 1], F32)
            nc.sync.dma_start(out=bkv_s, in_=bkv)
            mblk_s = wpool.tile([128, 128], F32)
            nc.sync.dma_start(out=mblk_s, in_=mblk)
            ones_s = wpool.tile([1, 64], f32r)
            nc.sync.dma_start(out=ones_s, in_=onesd[0:1, 0:64])
            ident_s = wpool.tile([128, 64], f32r)
            nc.sync.dma_start(out=ident_s, in_=ident)

            # ---- persistent activations ----
            qT_s = big.tile([128, 2, S], f32r)      # q^T, head-pair major
            kvT_s = big.tile([128, S], f32r)        # rows 0:64 k^T, 64:128 v^T
            k2_s = big.tile([128, S], f32r)         # k^T duplicated in both halves
            vaug_s = big.tile([128, NT, D + 1], f32r)  # v natural + ones col
            attn_s = big.tile([128, 2, S], f32r)    # normalized attn out^T

            nc.sync.dma_start(out=vaug_s[:, :, 64], in_=onesd[:, 0:NT])

            # ---- phase 1: projections q^T, k^T, v^T (per s-chunk) ----
            for c in range(NC):
                cs = slice(c * 512, (c + 1) * 512)
                psq = ps2.tile([128, 2, 512], F32, tag="ps2")
                pskv = ps1.tile([128, 512], F32, tag="ps1")
                for ib in range(2):
                    xt = xp.tile([128, 8, 512], f32r)
                    nc.sync.dma_start(
                        out=xt,
                        in_=xT[ib * 1024:(ib + 1) * 1024, cs]
                        .rearrange("(i p) s -> p i s", p=128))
                    for i8 in range(8):
                        i = ib * 8 + i8
                        st, sp = (i == 0), (i == NH - 1)
                        for j in range(2):
                            nc.tensor.matmul(
                                psq[:, j, :], wq_s[:, i, j * 128:(j + 1) * 128],
                                xt[:, i8, :], start=st, stop=sp)
                        nc.tensor.matmul(pskv, wkv_s[:, i, :], xt[:, i8, :],
                                         start=st, stop=sp)
                for j in range(2):
                    nc.vector.tensor_scalar_add(qT_s[:, j, cs], psq[:, j, :],
                                                bq_s[:, j:j + 1])
                nc.vector.tensor_scalar_add(kvT_s[:, cs], pskv, bkv_s)
                nc.vector.tensor_copy(k2_s[0:64, cs], kvT_s[0:64, cs])
                nc.vector.tensor_copy(k2_s[64:128, cs], kvT_s[0:64, cs])
                # v natural layout for the AV matmul, via PE transpose
                for t in range(4 * c, 4 * c + 4):
                    pst = ps1.tile([128, 512], F32, tag="ps1")
                    nc.tensor.transpose(
                        pst[0:128, 0:64].bitcast(f32r),
                        kvT_s[64:128, t * 128:(t + 1) * 128],
                        ident_s[64:128, :])
                    nc.vector.tensor_copy(vaug_s[:, t, 0:64],
                                          pst[0:128, 0:64].bitcast(f32r))

            nc.sync.dma_start(out=wo_s, in_=wo.rearrange("(j p) n -> p j n", p=128))

            # ---- phase 2: attention per (s-chunk, head) ----
            for c in range(NC):
                cs = slice(c * 512, (c + 1) * 512)
                n_t = 4 * (c + 1) if causal else NT
                if not causal:
                    mf_tiles = []
                    for t2 in range(0, NT, 2):
                        mt = mfp.tile([128, 2, 512], mybir.dt.bfloat16, tag="mf")
                        nc.sync.dma_start(
                            out=mt,
                            in_=mfull[t2 * 128:(t2 + 2) * 128, cs]
                            .rearrange("(w p) s -> p w s", p=128))
                        mf_tiles.append(mt)
                for h in range(R):
                    hp, jj = h % 2, h // 2
                    hsl = slice(hp * 64, hp * 64 + 64)
                    av = ps1.tile([128, 512], F32, tag="ps1")
                    for w in range(n_t // 2):
                        sc = ps2.tile([128, 2, 512], F32, tag="ps2")
                        pt = ptp.tile([128, 2, 512], f32r)
                        for k in range(2):
                            t = 2 * w + k
                            off = max(0, t * 128 - c * 512) if causal else 0
                            nc.tensor.matmul(
                                sc[:, k, off:512],
                                k2_s[hsl, t * 128:(t + 1) * 128],
                                qT_s[hsl, jj, c * 512 + off:(c + 1) * 512],
                                start=True, stop=True)
                            if causal and t >= 4 * c:
                                nc.vector.tensor_add(
                                    sc[:, k, off:off + 128],
                                    sc[:, k, off:off + 128], mblk_s)
                            elif not causal:
                                nc.vector.tensor_add(
                                    sc[:, k, :], sc[:, k, :],
                                    mf_tiles[w][:, k, :])
                        nc.scalar.activation(pt, sc,
                                             mybir.ActivationFunctionType.Exp)
                        for k in range(2):
                            t = 2 * w + k
                            off = max(0, t * 128 - c * 512) if causal else 0
                            nc.tensor.matmul(
                                av[0:65, off:512], vaug_s[:, t, :],
                                pt[:, k, off:512],
                                start=(t == 0), stop=(t == n_t - 1))
                    # normalize: out^T[d, s] * (1 / rowsum[s])
                    rs1 = rbp.tile([1, 512], f32r, tag="rs1")
                    nc.vector.tensor_copy(rs1, av[64:65, :])
                    rbps = ps1.tile([128, 512], F32, tag="ps1")
                    nc.tensor.matmul(rbps[0:64, :], ones_s, rs1,
                                     start=True, stop=True)
                    rb = rbp.tile([64, 512], F32, tag="rb")
                    nc.vector.reciprocal(rb, rbps[0:64, :])
                    nc.vector.tensor_mul(attn_s[hsl, jj, cs], av[0:64, :], rb)
                # ---- output projection for this chunk ----
                for st_ in range(4):
                    s0 = c * 512 + st_ * 128
                    op_s = opp.tile([128, H], F32, tag="op")
                    for n in range(4):
                        po = ps2.tile([128, 2, 512], F32, tag="ps2")
                        for j in range(2):
                            nc.tensor.matmul(
                                po[:, 0, :], attn_s[:, j, s0:s0 + 128],
                                wo_s[:, j, n * 512:(n + 1) * 512],
                                start=(j == 0), stop=(j == 1))
                        nc.vector.tensor_copy(op_s[:, n * 512:(n + 1) * 512],
                                              po[:, 0, :])
                    nc.sync.dma_start(out=partial[s0:s0 + 128, :], in_=op_s)

    _split_excess_waits(nc)
    return nc


def _get_program(causal: bool):
    if causal not in _PROGRAM_CACHE:
        _PROGRAM_CACHE[causal] = _build(causal)
    return _PROGRAM_CACHE[causal]


def kernel(x, causal_mask, Wq, bq, Wk, bk, Wv, bv, Wo, bo, _trace=False):
    x = np.asarray(x, dtype=np.float32)
    causal_mask = np.asarray(causal_mask, dtype=np.float32)
    Wq = np.asarray(Wq, dtype=np.float32)
    bq = np.asarray(bq, dtype=np.float32)
    Wk = np.asarray(Wk, dtype=np.float32)
    bk = np.asarray(bk, dtype=np.float32)
    Wv = np.asarray(Wv, dtype=np.float32)
    bv = np.asarray(bv, dtype=np.float32)
    Wo = np.asarray(Wo, dtype=np.float32)
    bo = np.asarray(bo, dtype=np.float32)

    xT = np.ascontiguousarray(x.reshape(S, H).T)
    causal = bool(
        np.array_equal(causal_mask,
                       np.triu(np.ones((S, S), np.float32), k=1)))
    scale = np.float32(1.0 / np.sqrt(D))
    mask_blk = (-1e9 * np.tril(np.ones((128, 128), np.float32), k=-1))
    identity = np.tile(np.eye(64, dtype=np.float32), (2, 1))

    in_maps = []
    for g in range(G):
        qsl = slice(g * HDG, (g + 1) * HDG)
        ksl = slice(g * D, (g + 1) * D)
        wq_g = np.ascontiguousarray(Wq[:, qsl] * scale)
        bq_g = np.ascontiguousarray(
            (bq[qsl] * scale).reshape(2, 128).T)           # [128, 2]
        wkv_g = np.ascontiguousarray(
            np.concatenate([Wk[:, ksl], Wv[:, ksl]], axis=1))
        bkv_g = np.ascontiguousarray(
            np.concatenate([bk[ksl], bv[ksl]]).reshape(128, 1))
        wo_g = np.ascontiguousarray(Wo[qsl, :])
        m = {
            "xT": xT, "wq": wq_g, "bq": bq_g, "wkv": wkv_g,
            "bkv": bkv_g, "wo": wo_g, "mblk": mask_blk, "ident": identity,
            "onesd": np.ones((128, 128), dtype=np.float32),
        }
        if not causal:
            import ml_dtypes
            m["mfull"] = np.ascontiguousarray(
                (causal_mask.T * np.float32(-1e9)).astype(ml_dtypes.bfloat16))
        in_maps.append(m)

    nc = _get_program(causal)
    res = run_bass_kernel_spmd(nc, in_maps, list(range(G)), trace=_trace)
    out = res.results[0]["partial"].astype(np.float32)
    for g in range(1, G):
        out = out + res.results[g]["partial"]
    out = out + bo[None, :]
    return out.reshape(1, S, H).astype(np.float32)


if __name__ == "__main__":
    rng = np.random.default_rng(0)
    pass

